# revision 56
# baseline (speedup 1.0000x reference)
"""GAT-with-gate kernel for Trainium2 (8 NeuronCores), v2.

Row-shards the 8192 receivers across 8 cores (1024 each). Per core:
  phase 1: h^T = W x^T + b  (bf16 matmul, f32 psum) -> hTp fp8 pairs
           h-rows (unbiased) -> haug bf16 [64][128, 258] (ones col for Z)
  phase 2: g^T = (W^T A_sym)^T x_loc^T + A_sym b -> gtp fp8 pairs
  phase 3: per 128-j block: e^T[j,i] = hTp^T gtp  (fp8 DoubleRow, 2x128
           contraction in one pass, 0.5 cyc/row)
           p = exp(e^T - 20) bf16 (ACT, psum->sbuf)
           p *= maskT (dense bf16 adjacency tile DMA'd from host) (DVE)
           agg: hacc[i] += p^T-block @ haug  (bf16 matmul, 16-block psum
           accumulation groups; hacc adds on GPSIMD)
  phase 4: h' = relu(hacc/Z + b); coeff = sigmoid([x,h'] gate); mix; store
"""
import os
import sys

import numpy as np

for _p in ("/opt/trn_rl_repo", "/root/.axon_site/_ro/trn_rl_repo"):
    if os.path.isdir(_p) and _p not in sys.path:
        sys.path.append(_p)

import ml_dtypes  # noqa: E402

import concourse.bass as bass  # noqa: E402
import concourse.mybir as mybir  # noqa: E402
import concourse.tile as tile  # noqa: E402
from concourse import bacc, library_config  # noqa: E402
from concourse.bass_utils import run_bass_kernel_spmd  # noqa: E402

N = 8192
D = 256
M = 8          # cores
NL = N // M    # 1024 local receivers per core
P = 128
JBLK = N // P  # 64 j-blocks
ICH = NL // P  # 8 local i-chunks
G = 16         # j-blocks per superblock
NSB = JBLK // G
DA = D + 2     # [h | 1 | pad]
EXP_SHIFT = -20.0

F32 = mybir.dt.float32
BF16 = mybir.dt.bfloat16
FP8 = mybir.dt.float8e4
AF = mybir.ActivationFunctionType
ALU = mybir.AluOpType
DR = mybir.MatmulPerfMode.DoubleRow

BF16NP = ml_dtypes.bfloat16

_BUILD_CACHE = {}

LAST_RESULT = None


def _build(gb):
    nc = bacc.Bacc(None, target_bir_lowering=False)

    xT_d = nc.dram_tensor("xT", (D, N), BF16, kind="ExternalInput")
    xtl_d = nc.dram_tensor("xtl", (D, NL), BF16, kind="ExternalInput")
    xloc_d = nc.dram_tensor("xloc", (P, ICH, D), F32, kind="ExternalInput")
    # wpack: [wst0 | wst1 | wgt0 | wgt1], each [128, 256] bf16
    wpack_d = nc.dram_tensor("wpack", (P, 4, D), BF16, kind="ExternalInput")
    # fpack: [bcol0, bcol1, bg0, bg1, brow(256), gwx(256), gwh(256)] f32
    fpack_d = nc.dram_tensor("fpack", (P, 4 + 3 * D), F32, kind="ExternalInput")
    # mask packed 4 j-blocks per tile row-set
    mask_d = nc.dram_tensor("mask", (JBLK // 4 * P, 4 * NL), BF16,
                            kind="ExternalInput")
    out_d = nc.dram_tensor("out", (P, ICH, D), F32, kind="ExternalOutput")

    with tile.TileContext(nc) as tc:
        with (
            tc.tile_pool(name="const", bufs=1) as cp,
            tc.tile_pool(name="maskp", bufs=3) as maskp,
            tc.tile_pool(name="escp", bufs=2) as escp,
            tc.tile_pool(name="work", bufs=3) as wp,
            tc.tile_pool(name="small", bufs=4) as smallp,
            tc.tile_pool(name="psmm", bufs=2, space="PSUM") as psmm,
            tc.tile_pool(name="psagg", bufs=3, space="PSUM") as psagg,
        ):
            nc.gpsimd.load_library(library_config.standard)

            # ---- persistent tiles ----
            fpack = cp.tile([P, 4 + 3 * D], F32, tag="fpack")
            bcol = [fpack[:, k:k + 1] for k in range(2)]
            bg = [fpack[:, 2 + k:3 + k] for k in range(2)]
            brow_b = fpack[:, 4:4 + D]
            gwx_b = fpack[:, 4 + D:4 + 2 * D]
            gwh_b = fpack[:, 4 + 2 * D:4 + 3 * D]
            hTp = cp.tile([P, 2, N], FP8, tag="hTp")
            gtp = cp.tile([P, 2, NL], FP8, tag="gtp")
            haug = cp.tile([P, JBLK, DA], BF16, tag="haug")
            hacc = [cp.tile([P, DA], F32, tag=f"hacc{i}", name=f"hacc{i}")
                    for i in range(ICH)]
            ebase = cp.tile([P, NL], BF16, tag="ebase")
            gbt = cp.tile([P, 1], F32, tag="gbt")
            xlp = cp.tile([P, ICH, D], F32, tag="xlp")
            otp = cp.tile([P, ICH, D], F32, tag="otp")
            sxs = [cp.tile([P, 1], F32, tag=f"sx{i}", name=f"sx{i}")
                   for i in range(ICH)]

            nc.vector.memset(haug[:, :, D:DA], 1.0)
            nc.vector.memset(ebase[:], float(np.e))
            nc.vector.memset(gbt[:], float(gb))

            # ---- phases 1-2 in a scoped pool (xT/weights freed before ph3) ----
            with tc.tile_pool(name="ph1", bufs=1) as ph1:
                xTb = [ph1.tile([P, N], BF16, tag=f"xT{k}", name=f"xT{k}")
                       for k in range(2)]
                xtl = [ph1.tile([P, NL], BF16, tag=f"xtl{k}", name=f"xtl{k}")
                       for k in range(2)]
                wpack = ph1.tile([P, 4, D], BF16, tag="wpack")
                wst = [wpack[:, k, :] for k in range(2)]
                wgt = [wpack[:, 2 + k, :] for k in range(2)]
                # weights first (small), then x in 4 chunks per half so the
                # first h matmul starts after ~1/4 of the x transfer
                nc.sync.dma_start(wpack[:], wpack_d[:])
                nc.sync.dma_start(fpack[:], fpack_d[:])
                xchunks = [(0, 512), (512, 512), (1024, 1024), (2048, 2048),
                           (4096, 2048), (6144, 2048)]
                for off, ln in xchunks:
                    sl = slice(off, off + ln)
                    for k in range(2):
                        nc.sync.dma_start(xTb[k][:, sl], xT_d[P * k:P * k + P, sl])

                # ---- phase 1: h^T -> hTp fp8 pairs; h rows -> haug bf16 ----
                # h psum spans two 512-col chunks: each ACT drain is
                # [128,1024], halving the per-drain fixed cost
                for jc in range(16):
                    sl = slice(512 * jc, 512 * jc + 512)
                    if jc % 2 == 0:
                        pss = [psmm.tile([P, NL], F32, tag="mm", name=f"hps{dc}_{jc}")
                               for dc in range(2)]
                    for dc in range(2):
                        half = jc % 2
                        for k in range(2):
                            nc.tensor.matmul(
                                pss[dc][:, 512 * half:512 * half + 512],
                                wst[k][:, P * dc:P * dc + P], xTb[k][:, sl],
                                start=(k == 0), stop=(k == 1),
                            )
                        if half == 1:
                            dsl = slice(1024 * (jc // 2), 1024 * (jc // 2) + 1024)
                            nc.scalar.activation(
                                hTp[:, dc, dsl], pss[dc][:], AF.Identity,
                                bias=bcol[dc][:], scale=1.0,
                            )
                    for sub in range(4):
                        jb = 4 * jc + sub
                        ps2 = psagg.tile([P, D], F32, tag="agg")
                        for k in range(2):
                            nc.tensor.matmul(
                                ps2[:], xTb[k][:, P * jb:P * jb + P], wst[k][:],
                                start=(k == 0), stop=(k == 1),
                            )
                        # haug rows carry the bias: sum_j p (h_j + b) / Z
                        # = h'pre/Z + b, so phase 4 is a pure relu(scale=1/Z)
                        nc.vector.tensor_tensor(haug[:, jb, 0:D], ps2[:],
                                                brow_b[:], op=ALU.add)

                # deferred small DMAs
                for k in range(2):
                    nc.sync.dma_start(xtl[k][:], xtl_d[P * k:P * k + P, :])

                # ---- phase 2: g^T -> gtp fp8 pairs ----
                for dc in range(2):
                    for ih in range(2):
                        sl = slice(512 * ih, 512 * ih + 512)
                        ps = psmm.tile([P, 512], F32, tag="mm")
                        for k in range(2):
                            nc.tensor.matmul(
                                ps[:], wgt[k][:, P * dc:P * dc + P], xtl[k][:, sl],
                                start=(k == 0), stop=(k == 1),
                            )
                        nc.scalar.activation(
                            gtp[:, dc, sl], ps[:], AF.Identity, bias=bg[dc][:],
                            scale=1.0,
                        )

            # ---- gate x-half (overlaps) ----
            nc.sync.dma_start(xlp[:], xloc_d[:])
            for ic in range(ICH):
                scr = wp.tile([P, D], F32, tag="scr")
                nc.vector.tensor_tensor(scr[:], xlp[:, ic, :], gwx_b[:], op=ALU.mult)
                nc.vector.reduce_sum(sxs[ic][:], scr[:], axis=mybir.AxisListType.X)

            # ---- phase 3: e^T (fp8 DR), exp, mask, aggregate ----
            # Variable superblocks: the last two are 8 blocks long so the
            # final (un-overlapped) aggregation batch is half-sized. Agg
            # chains for superblock k-1 are interleaved into superblock k's
            # per-block slots so PE fills ACT-exp gaps.
            SBS = [6, 10, 12, 12, 12, 12]
            SBO = [0, 6, 16, 28, 40, 52]

            def agg_chain(k, pts, ic, glo=0, ghi=None, first=None):
                psa = psagg.tile([P, DA], F32, tag="agg")
                ng = SBS[k] if ghi is None else ghi
                if first is None:
                    first = (k == 0 and glo == 0)
                for g in range(glo, ng):
                    jb = SBO[k] + g
                    nc.tensor.matmul(
                        psa[:], pts[g][:, P * ic:P * ic + P],
                        haug[:, jb, :],
                        start=(g == glo), stop=(g == ng - 1),
                    )
                if first:
                    nc.vector.tensor_copy(hacc[ic][:], psa[:])
                else:
                    nc.vector.tensor_tensor(
                        hacc[ic][:], psa[:], hacc[ic][:], op=ALU.add
                    )

            def phase4(ic):
                zrec = smallp.tile([P, 1], F32, tag="zrec")
                nc.vector.reciprocal(zrec[:], hacc[ic][:, D:D + 1])
                hp = wp.tile([P, D], F32, tag="hp")
                nc.scalar.activation(hp[:], hacc[ic][:, 0:D], AF.Relu,
                                     bias=0.0, scale=zrec[:])
                scr2 = wp.tile([P, D], F32, tag="scr")
                sh = smallp.tile([P, 1], F32, tag="sh")
                nc.gpsimd.tensor_tensor(scr2[:], hp[:], gwh_b[:], op=ALU.mult)
                nc.vector.reduce_sum(sh[:], scr2[:], axis=mybir.AxisListType.X)
                st = smallp.tile([P, 1], F32, tag="st")
                nc.gpsimd.tensor_tensor(st[:], sxs[ic][:], sh[:], op=ALU.add)
                cf = smallp.tile([P, 1], F32, tag="cf")
                nc.scalar.activation(cf[:], st[:], AF.Sigmoid,
                                     bias=gbt[:], scale=1.0)
                dif = wp.tile([P, D], F32, tag="scr")
                nc.gpsimd.tensor_tensor(dif[:], xlp[:, ic, :], hp[:],
                                        op=ALU.subtract)
                nc.vector.scalar_tensor_tensor(
                    out=otp[:, ic, :], in0=dif[:], scalar=cf[:],
                    in1=hp[:], op0=ALU.mult, op1=ALU.add,
                )
                nc.sync.dma_start(out_d[:, ic, :], otp[:, ic, :])

            with tc.tile_pool(name="ptp", bufs=2) as ptp:
                prev = None
                for k, ng in enumerate(SBS):
                    pts = [ptp.tile([P, NL], BF16, tag=f"pt{g}", name=f"pt{g}_{k}")
                           for g in range(ng)]
                    step = max(1, ng // 8)
                    for g in range(ng):
                        jb = SBO[k] + g
                        if jb % 4 == 0:
                            mask_t = maskp.tile([P, 4, NL], BF16, tag="mask")
                            nc.sync.dma_start(
                                mask_t[:], mask_d[P * (jb // 4):P * (jb // 4) + P, :]
                            )
                        mk = mask_t[:, jb % 4, :]
                        ps = psmm.tile([P, NL], F32, tag="mm")
                        for c4 in range(4):
                            nc.tensor.matmul(
                                ps[:, 256 * c4:256 * c4 + 256],
                                hTp[:, :, P * jb:P * jb + P],
                                gtp[:, :, 256 * c4:256 * c4 + 256],
                                start=True, stop=True, perf_mode=DR,
                            )
                        if jb % 16 >= 14 or jb in (3, 7):
                            # ACT relief path: DVE drains raw e to SBUF (f32),
                            # GPSIMD does the exp (no PSUM port on GPSIMD)
                            esc = escp.tile([P, NL], F32, tag="esc")
                            nc.vector.tensor_copy(esc[:], ps[:])
                            nc.gpsimd.tensor_tensor(pts[g][:], ebase[:], esc[:],
                                                    op=ALU.pow)
                        else:
                            nc.scalar.activation(pts[g][:], ps[:], AF.Exp,
                                                 bias=0.0, scale=1.0)
                        if (jb % 2 == 1) if jb < 32 else (jb % 3 == 1):
                            nc.gpsimd.tensor_tensor(pts[g][:], pts[g][:],
                                                    mk, op=ALU.mult)
                        else:
                            nc.vector.tensor_tensor(pts[g][:], pts[g][:],
                                                    mk, op=ALU.mult)
                        if prev is not None:
                            for ic in range(g * ICH // ng,
                                            (g + 1) * ICH // ng):
                                agg_chain(k - 1, prev, ic)
                    prev = pts
                # drain last superblock's aggregation + phase 4 per i-chunk
                for ic in range(ICH):
                    agg_chain(len(SBS) - 1, prev, ic)
                    phase4(ic)

    nc.compile()
    return nc


def _prep_mask(edge_index):
    """Dense adjacency (with self loops), per-core transposed bf16 slices:
    maskT_c[j, i] = adj[c*NL + i, j]."""
    adj = np.zeros((N, N), dtype=np.uint8)
    s = np.asarray(edge_index[0], dtype=np.int64)
    d = np.asarray(edge_index[1], dtype=np.int64)
    adj[s, d] = 1
    idx = np.arange(N)
    adj[idx, idx] = 1
    masks = []
    for c in range(M):
        sl = adj[c * NL:(c + 1) * NL, :].T.astype(BF16NP)
        masks.append(np.ascontiguousarray(sl))
    return masks


def prepare(x, edge_index, W_w, W_b, A, gate_w, gate_b):
    x = np.ascontiguousarray(np.asarray(x, dtype=np.float32))
    W_w = np.asarray(W_w, dtype=np.float32)
    W_b = np.asarray(W_b, dtype=np.float32)
    A = np.asarray(A, dtype=np.float32)
    gate_w = np.asarray(gate_w, dtype=np.float32)
    gb = float(np.asarray(gate_b).reshape(-1)[0])
    assert x.shape == (N, D)

    masks = _prep_mask(edge_index)

    key = (gb,)
    if key not in _BUILD_CACHE:
        _BUILD_CACHE[key] = _build(gb)
    nc = _BUILD_CACHE[key]

    xT = np.ascontiguousarray(x.T.astype(BF16NP))
    wstT = W_w.T.astype(BF16NP)                       # [d' , d]
    asym = (A + A.T).astype(np.float32)
    wgtT = (W_w.T @ asym).astype(BF16NP)
    # wpack [128, 4, 256]: [wst0 | wst1 | wgt0 | wgt1]
    wpack = np.ascontiguousarray(np.stack(
        [wstT[:P], wstT[P:], wgtT[:P], wgtT[P:]], axis=1))
    # fpack [128, 4+3*256] f32: bcol0 bcol1 bg0 bg1 brow gwx gwh (broadcast)
    bgc = (asym.T @ W_b).astype(np.float32)
    fpack = np.zeros((P, 4 + 3 * D), np.float32)
    fpack[:, 0] = W_b[:P]
    fpack[:, 1] = W_b[P:]
    fpack[:, 2] = bgc[:P]
    fpack[:, 3] = bgc[P:]
    fpack[:, 4:4 + D] = W_b[None, :]
    fpack[:, 4 + D:4 + 2 * D] = gate_w[:, :D]
    fpack[:, 4 + 2 * D:4 + 3 * D] = gate_w[:, D:]
    fpack = np.ascontiguousarray(fpack)

    in_maps = []
    for c in range(M):
        xl = x[c * NL:(c + 1) * NL]
        # mask packed: [16, 128, 4, 1024] -> [16*128, 4096]
        mp = masks[c].reshape(JBLK // 4, 4, P, NL).transpose(0, 2, 1, 3)
        mp = np.ascontiguousarray(mp.reshape(JBLK // 4 * P, 4 * NL))
        in_maps.append(dict(
            xT=xT,
            xtl=np.ascontiguousarray(xl.T.astype(BF16NP)),
            xloc=np.ascontiguousarray(
                xl.reshape(ICH, P, D).transpose(1, 0, 2)),
            wpack=wpack, fpack=fpack,
            mask=mp,
        ))
    return nc, in_maps


def kernel(x, edge_index, W_w, W_b, A, gate_w, gate_b):
    global LAST_RESULT
    nc, in_maps = prepare(x, edge_index, W_w, W_b, A, gate_w, gate_b)
    os.environ["BASS_NEVER_TRACE"] = "1"
    res = run_bass_kernel_spmd(nc, in_maps, core_ids=list(range(M)))
    LAST_RESULT = res
    out = np.concatenate(
        [res.results[c]["out"].transpose(1, 0, 2).reshape(NL, D)
         for c in range(M)], axis=0)
    return out


# revision 57
# speedup vs baseline: 1.0164x; 1.0164x over previous
"""GAT-with-gate kernel for Trainium2 (8 NeuronCores), v3.

Row-shards the 8192 receivers across 8 cores (1024 each). Per core:
  phase 1: h^T = W x^T + b (bf16 matmul, f32 psum) -> hTp fp8e4m3 pair-tile
           [128, 2, 8192]; h-row blocks (+bias) -> haug bf16 [128, 64, 258]
           (ones column accumulates the softmax denominator Z)
  phase 2: g^T = (W^T A_sym)^T x_loc^T + A_sym b -> gtp fp8 pairs (host folds
           A_sym = A + A^T into the weights, so e + e^T needs no transpose)
  phase 3: per 128-source block jb:
           e^T[j,i] = hTp^T gtp   (fp8 DoubleRow matmul: 2x128 contraction
                                   per pass at 0.5 cycles/row = 4x f32r)
           p = exp(e^T) bf16      (ACT drains psum; the usual max-shift
                                   cancels between numerator and Z, and
                                   exp(e) stays in f32/bf16 range)
           p *= maskT             (dense bf16 adjacency tiles streamed from
                                   HBM; multiply split DVE/GPSIMD)
           agg chains: hacc[ic] += p-block^T @ haug  (bf16 matmul, variable
           superblocks [6,10,12,12,12,12]; chains for superblock k-1 are
           interleaved into k's per-block slots so PE fills exp gaps)
           a few blocks run exp as DVE-drain + GPSIMD pow (ACT relief);
           GPSIMD has no PSUM port so it never reads psum directly
  phase 4: h' = relu((hacc/Z)); bias arrives via haug; coeff =
           sigmoid([x,h'] gate); out = coeff*x + (1-coeff)*h'; per-i-chunk
           output DMAs right after each final agg chain
"""
import os
import sys

import numpy as np

for _p in ("/opt/trn_rl_repo", "/root/.axon_site/_ro/trn_rl_repo"):
    if os.path.isdir(_p) and _p not in sys.path:
        sys.path.append(_p)

import ml_dtypes  # noqa: E402

import concourse.bass as bass  # noqa: E402
import concourse.mybir as mybir  # noqa: E402
import concourse.tile as tile  # noqa: E402
from concourse import bacc, library_config  # noqa: E402
from concourse.bass_utils import run_bass_kernel_spmd  # noqa: E402

N = 8192
D = 256
M = 8          # cores
NL = N // M    # 1024 local receivers per core
P = 128
JBLK = N // P  # 64 j-blocks
ICH = NL // P  # 8 local i-chunks
G = 16         # j-blocks per superblock
NSB = JBLK // G
DA = D + 2     # [h | 1 | pad]

F32 = mybir.dt.float32
BF16 = mybir.dt.bfloat16
FP8 = mybir.dt.float8e4
AF = mybir.ActivationFunctionType
ALU = mybir.AluOpType
DR = mybir.MatmulPerfMode.DoubleRow

BF16NP = ml_dtypes.bfloat16

_BUILD_CACHE = {}

LAST_RESULT = None


def _build(gb):
    nc = bacc.Bacc(None, target_bir_lowering=False)

    xT_d = nc.dram_tensor("xT", (D, N), BF16, kind="ExternalInput")
    xtl_d = nc.dram_tensor("xtl", (D, NL), BF16, kind="ExternalInput")
    xloc_d = nc.dram_tensor("xloc", (P, ICH, D), F32, kind="ExternalInput")
    # wpack: [wst0 | wst1 | wgt0 | wgt1], each [128, 256] bf16
    wpack_d = nc.dram_tensor("wpack", (P, 4, D), BF16, kind="ExternalInput")
    # fpack: [bcol0, bcol1, bg0, bg1, brow(256), gwx(256), gwh(256)] f32
    fpack_d = nc.dram_tensor("fpack", (P, 4 + 3 * D), F32, kind="ExternalInput")
    # mask packed 4 j-blocks per tile row-set
    mask_d = nc.dram_tensor("mask", (JBLK // 4 * P, 4 * NL), BF16,
                            kind="ExternalInput")
    out_d = nc.dram_tensor("out", (P, ICH, D), F32, kind="ExternalOutput")

    with tile.TileContext(nc) as tc:
        with (
            tc.tile_pool(name="const", bufs=1) as cp,
            tc.tile_pool(name="maskp", bufs=3) as maskp,
            tc.tile_pool(name="escp", bufs=2) as escp,
            tc.tile_pool(name="work", bufs=3) as wp,
            tc.tile_pool(name="small", bufs=4) as smallp,
            tc.tile_pool(name="psmm", bufs=2, space="PSUM") as psmm,
            tc.tile_pool(name="psagg", bufs=3, space="PSUM") as psagg,
        ):
            nc.gpsimd.load_library(library_config.standard)

            # ---- persistent tiles ----
            fpack = cp.tile([P, 4 + 3 * D], F32, tag="fpack")
            bcol = [fpack[:, k:k + 1] for k in range(2)]
            bg = [fpack[:, 2 + k:3 + k] for k in range(2)]
            brow_b = fpack[:, 4:4 + D]
            gwx_b = fpack[:, 4 + D:4 + 2 * D]
            gwh_b = fpack[:, 4 + 2 * D:4 + 3 * D]
            hTp = cp.tile([P, 2, N], FP8, tag="hTp")
            gtp = cp.tile([P, 2, NL], FP8, tag="gtp")
            haug = cp.tile([P, JBLK, DA], BF16, tag="haug")
            hacc = [cp.tile([P, DA], F32, tag=f"hacc{i}", name=f"hacc{i}")
                    for i in range(ICH)]
            ebase = cp.tile([P, NL], BF16, tag="ebase")
            gbt = cp.tile([P, 1], F32, tag="gbt")
            xlp = cp.tile([P, ICH, D], F32, tag="xlp")
            otp = cp.tile([P, ICH, D], F32, tag="otp")
            sxs = [cp.tile([P, 1], F32, tag=f"sx{i}", name=f"sx{i}")
                   for i in range(ICH)]

            nc.vector.memset(haug[:, :, D:DA], 1.0)
            nc.vector.memset(ebase[:], float(np.e))
            nc.vector.memset(gbt[:], float(gb))

            # ---- phases 1-2 in a scoped pool (xT/weights freed before ph3) ----
            with tc.tile_pool(name="ph1", bufs=1) as ph1:
                xTb = [ph1.tile([P, N], BF16, tag=f"xT{k}", name=f"xT{k}")
                       for k in range(2)]
                xtl = [ph1.tile([P, NL], BF16, tag=f"xtl{k}", name=f"xtl{k}")
                       for k in range(2)]
                wpack = ph1.tile([P, 4, D], BF16, tag="wpack")
                wst = [wpack[:, k, :] for k in range(2)]
                wgt = [wpack[:, 2 + k, :] for k in range(2)]
                # weights first (small), then x in 4 chunks per half so the
                # first h matmul starts after ~1/4 of the x transfer
                nc.sync.dma_start(wpack[:], wpack_d[:])
                nc.sync.dma_start(fpack[:], fpack_d[:])
                xchunks = [(0, 512), (512, 512), (1024, 1024), (2048, 2048),
                           (4096, 2048), (6144, 2048)]
                for off, ln in xchunks:
                    sl = slice(off, off + ln)
                    for k in range(2):
                        nc.sync.dma_start(xTb[k][:, sl], xT_d[P * k:P * k + P, sl])

                # ---- phase 1: h^T -> hTp fp8 pairs; h rows -> haug bf16 ----
                for jc in range(16):
                    sl = slice(512 * jc, 512 * jc + 512)
                    for dc in range(2):
                        ps = psmm.tile([P, 512], F32, tag="mm")
                        for k in range(2):
                            nc.tensor.matmul(
                                ps[:], wst[k][:, P * dc:P * dc + P], xTb[k][:, sl],
                                start=(k == 0), stop=(k == 1),
                            )
                        nc.scalar.activation(
                            hTp[:, dc, sl], ps[:], AF.Identity,
                            bias=bcol[dc][:], scale=1.0,
                        )
                    for sub in range(4):
                        jb = 4 * jc + sub
                        ps2 = psagg.tile([P, D], F32, tag="agg")
                        for k in range(2):
                            nc.tensor.matmul(
                                ps2[:], xTb[k][:, P * jb:P * jb + P], wst[k][:],
                                start=(k == 0), stop=(k == 1),
                            )
                        # haug rows carry the bias: sum_j p (h_j + b) / Z
                        # = h'pre/Z + b, so phase 4 is a pure relu(scale=1/Z)
                        nc.vector.tensor_tensor(haug[:, jb, 0:D], ps2[:],
                                                brow_b[:], op=ALU.add)

                # deferred small DMAs
                for k in range(2):
                    nc.sync.dma_start(xtl[k][:], xtl_d[P * k:P * k + P, :])

                # ---- phase 2: g^T -> gtp fp8 pairs ----
                for dc in range(2):
                    for ih in range(2):
                        sl = slice(512 * ih, 512 * ih + 512)
                        ps = psmm.tile([P, 512], F32, tag="mm")
                        for k in range(2):
                            nc.tensor.matmul(
                                ps[:], wgt[k][:, P * dc:P * dc + P], xtl[k][:, sl],
                                start=(k == 0), stop=(k == 1),
                            )
                        nc.scalar.activation(
                            gtp[:, dc, sl], ps[:], AF.Identity, bias=bg[dc][:],
                            scale=1.0,
                        )

            # ---- gate x-half (overlaps) ----
            nc.sync.dma_start(xlp[:], xloc_d[:])
            for ic in range(ICH):
                scr = wp.tile([P, D], F32, tag="scr")
                nc.vector.tensor_tensor(scr[:], xlp[:, ic, :], gwx_b[:], op=ALU.mult)
                nc.vector.reduce_sum(sxs[ic][:], scr[:], axis=mybir.AxisListType.X)

            # ---- phase 3: e^T (fp8 DR), exp, mask, aggregate ----
            # Variable superblocks: the last two are 8 blocks long so the
            # final (un-overlapped) aggregation batch is half-sized. Agg
            # chains for superblock k-1 are interleaved into superblock k's
            # per-block slots so PE fills ACT-exp gaps.
            SBS = [6, 10, 12, 12, 12, 12]
            SBO = [0, 6, 16, 28, 40, 52]

            def agg_chain(k, pts, ic, glo=0, ghi=None, first=None):
                psa = psagg.tile([P, DA], F32, tag="agg")
                ng = SBS[k] if ghi is None else ghi
                if first is None:
                    first = (k == 0 and glo == 0)
                for g in range(glo, ng):
                    jb = SBO[k] + g
                    nc.tensor.matmul(
                        psa[:], pts[g][:, P * ic:P * ic + P],
                        haug[:, jb, :],
                        start=(g == glo), stop=(g == ng - 1),
                    )
                if first:
                    nc.vector.tensor_copy(hacc[ic][:], psa[:])
                else:
                    nc.vector.tensor_tensor(
                        hacc[ic][:], psa[:], hacc[ic][:], op=ALU.add
                    )

            def phase4(ic):
                zrec = smallp.tile([P, 1], F32, tag="zrec")
                nc.vector.reciprocal(zrec[:], hacc[ic][:, D:D + 1])
                hp = wp.tile([P, D], F32, tag="hp")
                nc.scalar.activation(hp[:], hacc[ic][:, 0:D], AF.Relu,
                                     bias=0.0, scale=zrec[:])
                scr2 = wp.tile([P, D], F32, tag="scr")
                sh = smallp.tile([P, 1], F32, tag="sh")
                nc.gpsimd.tensor_tensor(scr2[:], hp[:], gwh_b[:], op=ALU.mult)
                nc.vector.reduce_sum(sh[:], scr2[:], axis=mybir.AxisListType.X)
                st = smallp.tile([P, 1], F32, tag="st")
                nc.gpsimd.tensor_tensor(st[:], sxs[ic][:], sh[:], op=ALU.add)
                cf = smallp.tile([P, 1], F32, tag="cf")
                nc.scalar.activation(cf[:], st[:], AF.Sigmoid,
                                     bias=gbt[:], scale=1.0)
                dif = wp.tile([P, D], F32, tag="scr")
                nc.gpsimd.tensor_tensor(dif[:], xlp[:, ic, :], hp[:],
                                        op=ALU.subtract)
                nc.vector.scalar_tensor_tensor(
                    out=otp[:, ic, :], in0=dif[:], scalar=cf[:],
                    in1=hp[:], op0=ALU.mult, op1=ALU.add,
                )
                nc.sync.dma_start(out_d[:, ic, :], otp[:, ic, :])

            with tc.tile_pool(name="ptp", bufs=2) as ptp:
                prev = None
                for k, ng in enumerate(SBS):
                    pts = [ptp.tile([P, NL], BF16, tag=f"pt{g}", name=f"pt{g}_{k}")
                           for g in range(ng)]
                    step = max(1, ng // 8)
                    for g in range(ng):
                        jb = SBO[k] + g
                        if jb % 4 == 0:
                            mask_t = maskp.tile([P, 4, NL], BF16, tag="mask")
                            nc.sync.dma_start(
                                mask_t[:], mask_d[P * (jb // 4):P * (jb // 4) + P, :]
                            )
                        mk = mask_t[:, jb % 4, :]
                        ps = psmm.tile([P, NL], F32, tag="mm")
                        for c4 in range(4):
                            nc.tensor.matmul(
                                ps[:, 256 * c4:256 * c4 + 256],
                                hTp[:, :, P * jb:P * jb + P],
                                gtp[:, :, 256 * c4:256 * c4 + 256],
                                start=True, stop=True, perf_mode=DR,
                            )
                        if jb % 16 >= 14 or jb in (3, 7):
                            # ACT relief path: DVE drains raw e to SBUF (f32),
                            # GPSIMD does the exp (no PSUM port on GPSIMD)
                            esc = escp.tile([P, NL], F32, tag="esc")
                            nc.vector.tensor_copy(esc[:], ps[:])
                            nc.gpsimd.tensor_tensor(pts[g][:], ebase[:], esc[:],
                                                    op=ALU.pow)
                        else:
                            nc.scalar.activation(pts[g][:], ps[:], AF.Exp,
                                                 bias=0.0, scale=1.0)
                        if (jb % 2 == 1) if jb < 32 else (jb % 3 == 1):
                            nc.gpsimd.tensor_tensor(pts[g][:], pts[g][:],
                                                    mk, op=ALU.mult)
                        else:
                            nc.vector.tensor_tensor(pts[g][:], pts[g][:],
                                                    mk, op=ALU.mult)
                        if prev is not None:
                            for ic in range(g * ICH // ng,
                                            (g + 1) * ICH // ng):
                                agg_chain(k - 1, prev, ic)
                    prev = pts
                # drain last superblock's aggregation + phase 4 per i-chunk
                for ic in range(ICH):
                    agg_chain(len(SBS) - 1, prev, ic)
                    phase4(ic)

    nc.compile()
    return nc


def _prep_mask(edge_index):
    """Dense adjacency (with self loops), per-core transposed bf16 slices:
    maskT_c[j, i] = adj[c*NL + i, j]."""
    adj = np.zeros((N, N), dtype=np.uint8)
    s = np.asarray(edge_index[0], dtype=np.int64)
    d = np.asarray(edge_index[1], dtype=np.int64)
    adj[s, d] = 1
    idx = np.arange(N)
    adj[idx, idx] = 1
    masks = []
    for c in range(M):
        sl = adj[c * NL:(c + 1) * NL, :].T.astype(BF16NP)
        masks.append(np.ascontiguousarray(sl))
    return masks


def prepare(x, edge_index, W_w, W_b, A, gate_w, gate_b):
    x = np.ascontiguousarray(np.asarray(x, dtype=np.float32))
    W_w = np.asarray(W_w, dtype=np.float32)
    W_b = np.asarray(W_b, dtype=np.float32)
    A = np.asarray(A, dtype=np.float32)
    gate_w = np.asarray(gate_w, dtype=np.float32)
    gb = float(np.asarray(gate_b).reshape(-1)[0])
    assert x.shape == (N, D)

    masks = _prep_mask(edge_index)

    key = (gb,)
    if key not in _BUILD_CACHE:
        _BUILD_CACHE[key] = _build(gb)
    nc = _BUILD_CACHE[key]

    xT = np.ascontiguousarray(x.T.astype(BF16NP))
    wstT = W_w.T.astype(BF16NP)                       # [d' , d]
    asym = (A + A.T).astype(np.float32)
    wgtT = (W_w.T @ asym).astype(BF16NP)
    # wpack [128, 4, 256]: [wst0 | wst1 | wgt0 | wgt1]
    wpack = np.ascontiguousarray(np.stack(
        [wstT[:P], wstT[P:], wgtT[:P], wgtT[P:]], axis=1))
    # fpack [128, 4+3*256] f32: bcol0 bcol1 bg0 bg1 brow gwx gwh (broadcast)
    bgc = (asym.T @ W_b).astype(np.float32)
    fpack = np.zeros((P, 4 + 3 * D), np.float32)
    fpack[:, 0] = W_b[:P]
    fpack[:, 1] = W_b[P:]
    fpack[:, 2] = bgc[:P]
    fpack[:, 3] = bgc[P:]
    fpack[:, 4:4 + D] = W_b[None, :]
    fpack[:, 4 + D:4 + 2 * D] = gate_w[:, :D]
    fpack[:, 4 + 2 * D:4 + 3 * D] = gate_w[:, D:]
    fpack = np.ascontiguousarray(fpack)

    in_maps = []
    for c in range(M):
        xl = x[c * NL:(c + 1) * NL]
        # mask packed: [16, 128, 4, 1024] -> [16*128, 4096]
        mp = masks[c].reshape(JBLK // 4, 4, P, NL).transpose(0, 2, 1, 3)
        mp = np.ascontiguousarray(mp.reshape(JBLK // 4 * P, 4 * NL))
        in_maps.append(dict(
            xT=xT,
            xtl=np.ascontiguousarray(xl.T.astype(BF16NP)),
            xloc=np.ascontiguousarray(
                xl.reshape(ICH, P, D).transpose(1, 0, 2)),
            wpack=wpack, fpack=fpack,
            mask=mp,
        ))
    return nc, in_maps


def kernel(x, edge_index, W_w, W_b, A, gate_w, gate_b):
    global LAST_RESULT
    nc, in_maps = prepare(x, edge_index, W_w, W_b, A, gate_w, gate_b)
    os.environ["BASS_NEVER_TRACE"] = "1"
    res = run_bass_kernel_spmd(nc, in_maps, core_ids=list(range(M)))
    LAST_RESULT = res
    out = np.concatenate(
        [res.results[c]["out"].transpose(1, 0, 2).reshape(NL, D)
         for c in range(M)], axis=0)
    return out
